# revision 38
# baseline (speedup 1.0000x reference)
"""MinHash sketch kernel for Trainium2 (8 NeuronCores, Bass/Tile).

Computes: sketch = segment_min(x @ hash_matrices.T, batch) over 512 segments,
with empty segments set to 0.  x: [N, 256] f32, batch: [N] sorted int64,
hash_matrices: [128, 256] f32 -> out [512, 128] f32.

Strategy (data-parallel over nodes):
  * Host sorts nodes by segment and cuts the order into W=64-wide windows,
    distributed contiguously over the 8 cores (padded with repeats of the
    last node - min-neutral - so every core runs the identical program).
    Windows that straddle a segment boundary (~num_segments of them) are
    recomputed exactly on the host and their device minima ignored.
  * Each core's node shard is laid out TRANSPOSED on host ([256, cols]) so the
    contraction dim (features) sits on SBUF partitions - no on-device
    transpose needed.
  * Device: stream x in 2048-column blocks; each 512-column PSUM bank's
    worth arrives as ONE folded [128, 2, 512] DMA (both 128-feature chunks),
    blocks alternating between the sync and scalar HWDGE queues so the
    ~650ns per-DMA issue cost never starves the DMA engines.  hv[h, n] is
    accumulated per bank in its own PSUM tile over the two feature chunks
    (h = 128 hashes on partitions), then one segmented reduce_min per bank
    ([128, 8, 64] -> [128, 8]) into an SBUF accumulator [128, G] (bf16),
    flushed progressively to DRAM on the SWDGE queue.
  * Tail: the sub-2048 remainder streams FIRST; the program ends with
    geometrically shrinking blocks (1024..64) so the serial
    DMA->sem->matmul->reduce->flush chain after the last byte is minimal.
  * Host: scatter-min each (core, group) column back to its segment, zero
    empty segments; exact host matmul for the boundary windows.
  * No collective needed: group->segment mapping is host-side, so per-core
    partial sketches are min-combined on the host during unsharding.
  * Cost model (TimelineSim): ~96.9 us/core vs ~90.3 us of pure DMA at the
    ~360 GB/s modeled HBM roof - DMA-bound with ~2us startup and ~4.5us
    of post-stream latency (sem props + reduce chain + flush pipeline).

Precision/speed scheme for the matmul (SCHEME):
  * "bf16":  x and H rounded to bf16 on host; single-term matmul at full PE
             rate and HALF the DMA bytes of fp32 (~4e-3 rel error, gate is
             2e-2).  ~32 MB/core of DMA.
  * "hilo":  x and H split into bf16 hi+lo pairs on host; 3-term product
             (hi*hi + hi*lo + lo*hi) at full PE rate.  ~4e-6 rel error,
             same DMA bytes as fp32 (~64 MB/core -> ~189 us).
  * "f32r":  x, H rounded to FP32R (1-8-11) on host; single-term matmul at
             full PE rate.  ~1.5e-4 rel error, fp32 DMA bytes.
"""

import sys

if "/opt/trn_rl_repo" not in sys.path:
    sys.path.insert(0, "/opt/trn_rl_repo")

import numpy as np

SCHEME = "bf16"
ACC_BF16 = True  # store/flush per-window minima as bf16 (halves output DMA)
N_CORES = 8
W = 64           # nodes per group (reduce_min granularity)
BANK = 512       # PSUM bank width (fp32)
SEG = 512        # PSUM tile width (one bank); one reduce per tile
TB = 2048        # columns per full DMA block
NUM_HASHES = 128
FEATURE_DIM = 256

_compiled_cache = {}


def round_fp32r(a):
    """Round-to-nearest-even to FP32R (1-8-11); low 12 mantissa bits zero."""
    b = np.ascontiguousarray(a, dtype=np.float32).view(np.uint32)
    low = b & np.uint32(0xFFF)
    b2 = b & np.uint32(0xFFFFF000)
    up = (low > 0x800) | ((low == 0x800) & (((b2 >> 12) & 1) == 1))
    return (b2 + (up.astype(np.uint32) << 12)).view(np.float32)


def _block_widths(cols):
    """The sub-TB remainder goes FIRST (its compute overlaps the stream) and
    the program ends with geometrically shrinking blocks, so the DVE reduce
    backlog after the final x DMA (matmul + reduce + final flush chain) is
    as short as possible.  Every tail block's reduce only starts after its
    own DMA + sem prop, so late blocks must be small."""
    tail = [1024, 512, 448, 64]                   # sum = 2048
    if cols <= TB + 2048:
        tail = [t for t in tail if t < cols]
        rest = cols - sum(tail)
        assert rest >= 0, (cols,)
        widths = ([rest] if rest else []) + tail
    else:
        # geometrically shrinking blocks at the end: the DVE reduce backlog
        # built up over the full blocks drains during the 1024/512 pieces
        # (same work/arrival ratio but the last FULL block sits ~4us before
        # the stream end), leaving only the tiny pieces' reduces in the
        # post-stream serial chain
        n_full, lead = divmod(cols - 2048, TB)
        widths = ([lead] if lead else []) + [TB] * n_full + tail
    assert sum(widths) == cols and all(0 < w <= TB and w % W == 0
                                       for w in widths), (cols, widths)
    return widths


def _build_program(cols, scheme):
    """Build + compile the single-core Bass program for a shard of `cols`
    node-columns (cols % W == 0)."""
    import concourse.bacc as bacc
    import concourse.mybir as mybir
    import concourse.tile as tile

    nc = bacc.Bacc("TRN2", target_bir_lowering=False, debug=False,
                   num_devices=N_CORES)

    assert cols % W == 0
    n_groups = cols // W
    widths = _block_widths(cols)

    if scheme == "hilo":
        xdt = mybir.dt.bfloat16
        x_names = ["xhi", "xlo"]
        h_names = ["hhi", "hlo"]
    elif scheme == "bf16":
        xdt = mybir.dt.bfloat16
        x_names = ["xt"]
        h_names = ["ht"]
    else:
        xdt = mybir.dt.float32r if scheme == "f32r" else mybir.dt.float32
        x_names = ["xt"]
        h_names = ["ht"]

    acc_dt = mybir.dt.bfloat16 if ACC_BF16 else mybir.dt.float32

    x_in = {n: nc.dram_tensor(n, [FEATURE_DIM, cols], xdt,
                              kind="ExternalInput").ap() for n in x_names}
    # h packed host-side as [128, 2*NUM_HASHES]: hpk[p, c*NH+j] = H[j, c*128+p]
    # -> single 512B-per-row DMA (>= 512B avoids the small-descriptor penalty)
    h_in = {n: nc.dram_tensor(n, [128, 2 * NUM_HASHES], xdt,
                              kind="ExternalInput").ap() for n in h_names}
    acc_out = nc.dram_tensor("acc", [NUM_HASHES, n_groups], acc_dt,
                             kind="ExternalOutput").ap()

    with tile.TileContext(nc) as tc:
        with (
            tc.tile_pool(name="singles", bufs=1) as singles,
            tc.tile_pool(name="xtiles", bufs=8) as xtiles,
            tc.tile_pool(name="psum", bufs=8, space="PSUM") as psum,
        ):
            acc_sb = singles.tile([128, n_groups], acc_dt)
            h_sb = {}
            for n in h_names:
                t = singles.tile([128, 2, NUM_HASHES], xdt, tag=f"h_{n}")
                # gpsimd queue: don't delay the first x block on the HWDGE queue
                nc.gpsimd.dma_start(out=t[:, :, :], in_=h_in[n][:, :])
                h_sb[n] = t

            # (weight tensor, chunk, rhs tensor) per accumulation term
            if scheme == "hilo":
                phases = [("hhi", 0, "xhi"), ("hhi", 1, "xhi"),
                          ("hlo", 0, "xhi"), ("hlo", 1, "xhi"),
                          ("hhi", 0, "xlo"), ("hhi", 1, "xlo")]
            else:
                phases = [(h_names[0], 0, x_names[0]),
                          (h_names[0], 1, x_names[0])]

            # x viewed as [p, c, n]: feature c*128+p, node-column n -- lets
            # one DMA move both 128-feature chunks of a block (each dma_start
            # costs ~650ns of sequencer issue time, which starved the DMA
            # engines during the many-small-blocks tail)
            x_pcn = {n: x_in[n].rearrange("(c p) n -> p c n", c=2)
                     for n in x_names}

            col0 = 0
            flushed = 0
            # few, large flushes: each range must be >= 256 groups so the
            # per-row transfer stays >= 512B (below that DMA pays a 2x
            # small-descriptor latency penalty)
            flush_step = max(2 * TB, cols // 2)
            flush_at = flush_step
            for bi, tb in enumerate(widths):
                n_banks = -(-tb // BANK)
                # one DMA per PSUM-bank's worth of columns (both feature
                # chunks folded into a single [128, 2, bw] transfer), so the
                # first bank's matmuls+reduce start after ~1/4 of the block
                # has landed -- this shrinks the DVE lag that otherwise sits
                # after the final x DMA.  Blocks alternate between the sync
                # and scalar queues to hide the per-DMA ~650ns issue cost.
                qeng = nc.sync if bi % 2 == 0 else nc.scalar
                x_sb = {}
                for n in x_names:
                    t = xtiles.tile([128, 2, TB], xdt, tag=f"x_{n}")
                    for k in range(n_banks):
                        ksl = slice(k * BANK, min((k + 1) * BANK, tb))
                        gsl = slice(col0 + ksl.start, col0 + ksl.stop)
                        qeng.dma_start(out=t[:, :, ksl],
                                       in_=x_pcn[n][:, :, gsl])
                    x_sb[n] = t

                # one PSUM tile per SEG=512 bank; matmuls fill it (all
                # phases), then ONE reduce drains it.  Separate tiles per
                # segment avoid false WAR hazards between a segment's reduce
                # and the next segment's matmuls; 8 bufs = all 8 banks give
                # two full blocks of pipelining.
                for s0 in range(0, tb, SEG):
                    sw = min(SEG, tb - s0)
                    hv = psum.tile([128, SEG], mybir.dt.float32, tag="hv")
                    for k in range(-(-sw // BANK)):
                        ksl = slice(s0 + k * BANK, min(s0 + (k + 1) * BANK,
                                                       s0 + sw))
                        psl = slice(k * BANK, k * BANK + ksl.stop - ksl.start)
                        for p, (hn, chunk, xn) in enumerate(phases):
                            nc.tensor.matmul(hv[:, psl],
                                             h_sb[hn][:, chunk, :],
                                             x_sb[xn][:, chunk, ksl],
                                             start=(p == 0),
                                             stop=(p == len(phases) - 1))
                    g0 = (col0 + s0) // W
                    nc.vector.tensor_reduce(
                        out=acc_sb[:, g0:g0 + sw // W],
                        in_=hv[:, :sw].rearrange("p (g w) -> p g w", w=W),
                        axis=mybir.AxisListType.X,
                        op=mybir.AluOpType.min,
                    )
                col0 += tb
                # flush finished accumulator ranges on the SWDGE queue (doesn't
                # block the HWDGE input stream); always flush after the last
                # full block, so the final flush is just the last tiny
                # blocks' groups (the SWDGE desc-gen latency would otherwise
                # sit in the post-final-DMA critical chain)
                if col0 < cols and (col0 >= flush_at or bi == len(widths) - 4):
                    g1 = col0 // W
                    if g1 > flushed:
                        nc.gpsimd.dma_start(out=acc_out[:, flushed:g1],
                                            in_=acc_sb[:, flushed:g1])
                        flushed = g1
                        flush_at = col0 + flush_step

            # final flush on HWDGE: the input queue is drained by now and
            # HWDGE first-byte latency is ~0.4us lower than SWDGE (one DMA:
            # a second one would serialize on the shared HWDGE generator)
            nc.sync.dma_start(out=acc_out[:, flushed:], in_=acc_sb[:, flushed:])

    nc.compile()
    return nc


def _pack_h(hm, bf16):
    """[NUM_HASHES, 256] f32 -> [128, 2*NUM_HASHES] with
    hpk[p, c*NH + j] = H[j, c*128 + p], so h_sb[p, c, j] lines up."""
    ht = hm.T                                    # [256, NH] (feat, hash)
    v = ht.reshape(2, 128, NUM_HASHES)           # [c, p, j]
    v = np.transpose(v, (1, 0, 2))               # [p, c, j]
    return np.ascontiguousarray(v.reshape(128, 2 * NUM_HASHES).astype(bf16))


def kernel(x, batch, num_segments, hash_matrices):
    import ml_dtypes
    from concourse import bass_utils

    x = np.ascontiguousarray(np.asarray(x), dtype=np.float32)
    batch = np.asarray(batch).astype(np.int64).ravel()
    num_segments = int(num_segments)
    hm = np.asarray(hash_matrices, dtype=np.float32)

    assert x.shape[1] == FEATURE_DIM and hm.shape == (NUM_HASHES, FEATURE_DIM)

    # --- host: window construction -----------------------------------------
    # Sort nodes by segment, pad to a uniform per-core column count with
    # repeats of the last node (same segment -> min-neutral), and cut the
    # order into fixed W-wide windows.  A window whose nodes all share one
    # segment is reduced on device; the ~num_segments windows that straddle
    # a segment boundary are recomputed exactly on the host (tiny).
    n_nodes = batch.shape[0]
    counts = np.bincount(batch, minlength=num_segments)
    order = np.argsort(batch, kind="stable")  # contiguous runs per segment

    gpc = -(-(-(-n_nodes // N_CORES)) // W)   # ceil(ceil(n/8)/W)
    cols = gpc * W
    n_pad = cols * N_CORES - n_nodes
    ord_pad = np.concatenate([order, np.full(n_pad, order[-1], dtype=np.int64)])
    idx = ord_pad.reshape(N_CORES, cols)

    bs = batch[ord_pad].reshape(N_CORES, gpc, W)   # sorted segment per slot
    pure = bs[:, :, 0] == bs[:, :, -1]
    grp_seg = np.where(pure, bs[:, :, 0], -1)      # [N_CORES, gpc]

    # --- host: build per-core shards ---------------------------------------
    bf16 = ml_dtypes.bfloat16
    in_maps = []
    if SCHEME == "bf16":
        hpk = _pack_h(hm, bf16)
        for c in range(N_CORES):
            xt = np.ascontiguousarray(x[idx[c]].T.astype(bf16))  # [256, cols]
            in_maps.append({"xt": xt, "ht": hpk})
    elif SCHEME == "hilo":
        hhi32 = hm.T.astype(bf16).astype(np.float32)
        hhi = _pack_h(hm, bf16)
        hlo = _pack_h((hm.T - hhi32).T, bf16)
        for c in range(N_CORES):
            xt = x[idx[c]].T                         # [256, cols] f32
            xhi = xt.astype(bf16)
            xlo = (xt - xhi.astype(np.float32)).astype(bf16)
            in_maps.append({"xhi": np.ascontiguousarray(xhi),
                            "xlo": np.ascontiguousarray(xlo),
                            "hhi": hhi, "hlo": hlo})
    else:
        dt = np.float32
        conv = round_fp32r if SCHEME == "f32r" else (
            lambda a: np.ascontiguousarray(a, dtype=dt))
        ht = conv(_pack_h(hm, np.float32).astype(np.float32))
        for c in range(N_CORES):
            in_maps.append({"xt": conv(np.ascontiguousarray(x[idx[c]].T)),
                            "ht": ht})

    # --- device ------------------------------------------------------------
    key = (cols, SCHEME, ACC_BF16)
    if key not in _compiled_cache:
        _compiled_cache[key] = _build_program(cols, SCHEME)
    nc = _compiled_cache[key]

    res = bass_utils.run_bass_kernel_spmd(
        nc, in_maps, core_ids=list(range(N_CORES)), trace=False
    )

    # --- host: combine -----------------------------------------------------
    sketch = np.full((num_segments, NUM_HASHES), np.inf, dtype=np.float32)
    for c in range(N_CORES):
        acc = np.asarray(res.results[c]["acc"]).astype(np.float32)  # [128, gpc]
        valid = grp_seg[c] >= 0
        np.minimum.at(sketch, grp_seg[c][valid], acc.T[valid])
    # exact host fixup for boundary (impure) windows
    fix_nodes = idx.reshape(N_CORES, gpc, W)[~pure].ravel()
    if fix_nodes.size:
        hv_fix = x[fix_nodes] @ hm.T               # [n_fix, 128] fp32
        np.minimum.at(sketch, batch[fix_nodes], hv_fix)
    sketch[counts == 0] = 0.0
    return sketch
